# revision 10
# baseline (speedup 1.0000x reference)
"""FNO2d classifier Trainium2 Bass kernel.

Sharding: pure data-parallel over batch (16 samples -> 8 cores x 2 samples).
All parameters replicated; per-core program identical (SPMD).

Algorithm (per core, per layer) — partial-DFT formulation (only 32x16 modes):
  lift:   x0 = gelu(grid @ lift_w.T + b)                        [2,64,256,256]
  A:      Xh = DFT_h(x) at 32 ky modes      (PE, per (s,c,wc))  psA [128w, 64=(XhR32|XhI32)]
  B:      X2 = DFT_w(Xh) at 16 kx modes     (PE, per (s,ky))    psB [64c, 32=(X2R16|X2I16)]
  C:      X3 = W_mode^T @ X2  (channel mix) (PE, per mode)      out [64o, 4=(var2,s2)] -> P_all
  C->D:   PE-transpose P_all kx-blocks  -> X3E [64=(rin,ky), (kx,o,var)]
  D:      T' = X3 @ Gh (inverse DFT over ky)(PE, per (s,o))     psD [32=(var,kx), 256h]
  E:      y = T' @ Gw + skip(x) + bias; gelu; +x (residual)     psY [64o, 512=2h*256w]
  proj+mean-pool, then two small MLP heads.

BatchNorm (eval) folded into C-weights / skip weights / bias on host.
f32r (tf32-like, ~2e-4 rel) used for wide matmuls; fp32 elsewhere.
"""
import sys
sys.path.insert(0, '/opt/trn_rl_repo')
import numpy as np

B, CIN, H, W = 16, 15, 256, 256
C, L, M = 64, 4, 16
ENV, D1D = 40, 4
HEAD_IN = C + ENV + D1D
BN_EPS = 1e-5
KY, KX = 32, 16
NCORES = 8
BS = B // NCORES          # 2 samples per core
HW = H * W
NMODE = KY * KX           # 512
WCHUNK = 16               # modes per weight DMA chunk
NPX = 512                 # pixels per lift/proj chunk


def _gelu_np(x):
    from scipy.special import erf
    return x * 0.5 * (1.0 + erf(x / np.sqrt(2.0)))


def precompute_consts(inp):
    """Host-side constant tensors (replicated to every core)."""
    ct = {}
    ky_vals = np.concatenate([np.arange(16), np.arange(240, 256)])
    kx_vals = np.arange(16)
    h = np.arange(H)
    w = np.arange(W)

    ang_h = 2 * np.pi * np.outer(h, ky_vals) / H            # [256, 32]
    FhT = np.concatenate([np.cos(ang_h), -np.sin(ang_h)], axis=1)   # [256, 64]
    ct['FhT'] = FhT.astype(np.float32)                      # [256, 64] (2 chunks of 128)

    ang_w = 2 * np.pi * np.outer(w, kx_vals) / W            # [256, 16]
    ct['FwR'] = np.concatenate([np.cos(ang_w), -np.sin(ang_w)], axis=1).astype(np.float32)  # [256,32]
    ct['FwI'] = np.concatenate([np.sin(ang_w), np.cos(ang_w)], axis=1).astype(np.float32)

    scale = inp['bn_g'] / np.sqrt(inp['bn_v'] + BN_EPS)     # [L, C]
    shift = inp['bn_b'] - inp['bn_m'] * scale
    ct['biasP'] = (inp['skip_b'] * scale + shift).astype(np.float32)               # [L, C]
    ct['skip_wT'] = np.einsum('loc,lo->lco', inp['skip_w'], scale).astype(np.float32)  # [L, c, o]

    # C weights, chunk-packed: W2[l, ch, p(128), m(16), o(64)]
    w1 = inp['w1r'] + 1j * inp['w1i']     # [L, i, o, 16, 16]
    w2 = inp['w2r'] + 1j * inp['w2i']
    Wall = np.concatenate([w1, w2], axis=3)        # [L, i, o, 32ky, 16kx]
    Wall = Wall * scale[:, None, :, None, None]    # fold bn scale on o
    # -> [L, ky, kx, 128=(iR|iI), o]
    Wd = np.concatenate([Wall.real, Wall.imag], axis=1)      # [L, 128, o, ky, kx]
    Wd = np.transpose(Wd, (0, 3, 4, 1, 2))                   # [L, ky, kx, 128, o]
    # reorder modes kx-major: mode' = kxg*128 + ky*4 + kxl  <->  (ky, kx=kxg*4+kxl)
    Wd = Wd.reshape(L, KY, 4, 4, 128, C)                     # [L, ky, kxg, kxl, 128, C]
    Wd = np.transpose(Wd, (0, 2, 1, 3, 4, 5))                # [L, kxg, ky, kxl, 128, C]
    Wd = Wd.reshape(L, NMODE, 128, C)
    ct['W2'] = np.ascontiguousarray(
        Wd.reshape(L, NMODE // WCHUNK, WCHUNK, 128, C).transpose(0, 1, 3, 2, 4)
    ).astype(np.float32)                                     # [L, 32, 128, 16, 64]

    GhD = np.concatenate([np.cos(ang_h.T), np.sin(ang_h.T)], axis=0)   # [64, 256]
    ct['GhD'] = GhD.astype(np.float32)

    c_kx = np.ones(KX); c_kx[1:] = 2.0
    GwR = (c_kx[:, None] / HW) * np.cos(ang_w.T)
    GwI = -(c_kx[:, None] / HW) * np.sin(ang_w.T)
    ct['GwE'] = np.concatenate([GwR, GwI], axis=0).astype(np.float32)  # [32, 256]

    ct['lift_wT'] = inp['lift_w'].T.astype(np.float32)       # [15, 64]
    ct['lift_b'] = inp['lift_b'].astype(np.float32)
    ct['proj_wT'] = inp['proj_w'].T.astype(np.float32)
    ct['proj_b'] = inp['proj_b'].astype(np.float32)

    dw1T = inp['dw1'].T.copy(); dw1T[0:C, :] /= HW
    iw1T = inp['iw1'].T.copy(); iw1T[0:C, :] /= HW
    ct['dw1T'] = dw1T.astype(np.float32)   # [108, 128]
    ct['iw1T'] = iw1T.astype(np.float32)
    ct['dw2T'] = inp['dw2'].T.astype(np.float32)
    ct['dw3T'] = inp['dw3'].T.astype(np.float32)
    ct['iw2T'] = inp['iw2'].T.astype(np.float32)
    ct['iw3T'] = inp['iw3'].T.astype(np.float32)
    for k in ('db1', 'db2', 'db3', 'ib1', 'ib2', 'ib3'):
        ct[k] = inp[k].astype(np.float32)
    return ct


_CACHE = {}


def build_program(debug_taps=False):
    import concourse.bass as bass
    import concourse.bacc as bacc
    import concourse.tile as tile
    from concourse import mybir
    from concourse.masks import make_identity

    F32 = mybir.dt.float32
    F32R = mybir.dt.float32r
    GELU = mybir.ActivationFunctionType.Gelu
    IDENT = mybir.ActivationFunctionType.Identity

    nc = bacc.Bacc(trn_type="TRN2", target_bir_lowering=False, debug=False,
                   num_devices=NCORES)

    # ---------------- DRAM I/O ----------------
    d_grid = nc.dram_tensor("grid", [BS, CIN, HW], F32R, kind="ExternalInput")
    d_env = nc.dram_tensor("env", [BS, ENV], F32, kind="ExternalInput")
    d_d1d = nc.dram_tensor("d1d", [BS, D1D], F32, kind="ExternalInput")
    d_FhT = nc.dram_tensor("FhT", [H, 64], F32R, kind="ExternalInput")
    d_FwR = nc.dram_tensor("FwR", [W, 32], F32R, kind="ExternalInput")
    d_FwI = nc.dram_tensor("FwI", [W, 32], F32R, kind="ExternalInput")
    d_W2 = nc.dram_tensor("W2", [L, NMODE // WCHUNK, 128, WCHUNK * C], F32, kind="ExternalInput")
    d_GhD = nc.dram_tensor("GhD", [64, H], F32R, kind="ExternalInput")
    d_GwE = nc.dram_tensor("GwE", [32, W], F32R, kind="ExternalInput")
    d_skipT = nc.dram_tensor("skip_wT", [L, C, C], F32R, kind="ExternalInput")
    d_biasP = nc.dram_tensor("biasP", [L, C], F32, kind="ExternalInput")
    d_liftT = nc.dram_tensor("lift_wT", [CIN, C], F32R, kind="ExternalInput")
    d_liftb = nc.dram_tensor("lift_b", [C], F32, kind="ExternalInput")
    d_projT = nc.dram_tensor("proj_wT", [C, C], F32R, kind="ExternalInput")
    d_projb = nc.dram_tensor("proj_b", [C], F32, kind="ExternalInput")
    d_dw1T = nc.dram_tensor("dw1T", [HEAD_IN, 128], F32, kind="ExternalInput")
    d_dw2T = nc.dram_tensor("dw2T", [128, 64], F32, kind="ExternalInput")
    d_dw3T = nc.dram_tensor("dw3T", [64, 8], F32, kind="ExternalInput")
    d_iw1T = nc.dram_tensor("iw1T", [HEAD_IN, 128], F32, kind="ExternalInput")
    d_iw2T = nc.dram_tensor("iw2T", [128, 64], F32, kind="ExternalInput")
    d_iw3T = nc.dram_tensor("iw3T", [64, 4], F32, kind="ExternalInput")
    d_db = [nc.dram_tensor(f"db{i}", [n], F32, kind="ExternalInput")
            for i, n in ((1, 128), (2, 64), (3, 8))]
    d_ib = [nc.dram_tensor(f"ib{i}", [n], F32, kind="ExternalInput")
            for i, n in ((1, 128), (2, 64), (3, 4))]

    d_odir = nc.dram_tensor("out_dir", [BS, 8], F32, kind="ExternalOutput")
    d_oint = nc.dram_tensor("out_int", [BS, 4], F32, kind="ExternalOutput")

    # internal ping-pong activations
    d_xA = nc.dram_tensor("xA", [BS, C, HW], F32R)
    d_xB = nc.dram_tensor("xB", [BS, C, HW], F32R)

    taps = {}
    if debug_taps:
        taps['x0'] = nc.dram_tensor("tap_x0", [BS, C, HW], F32R, kind="ExternalOutput")
        taps['RB'] = nc.dram_tensor("tap_RB", [128, NMODE * 4], F32, kind="ExternalOutput")
        taps['X3E'] = nc.dram_tensor("tap_X3E", [BS, 64, 2048], F32, kind="ExternalOutput")
        taps['Tp'] = nc.dram_tensor("tap_Tp", [BS, C, 32, H], F32, kind="ExternalOutput")
        taps['x1'] = nc.dram_tensor("tap_x1", [BS, C, HW], F32R, kind="ExternalOutput")

    with tile.TileContext(nc) as tc:
        import contextlib
        ctx = contextlib.ExitStack()
        with ctx:
            # ---- pools ----
            const_p = ctx.enter_context(tc.tile_pool(name="const", bufs=1))
            xt_p = ctx.enter_context(tc.tile_pool(name="xt", bufs=4))
            big_p = ctx.enter_context(tc.tile_pool(name="big", bufs=1))
            wsb_p = ctx.enter_context(tc.tile_pool(name="wsb", bufs=2))
            sb_p = ctx.enter_context(tc.tile_pool(name="sb", bufs=2))
            ep_p = ctx.enter_context(tc.tile_pool(name="ep", bufs=3))
            psAB_p = ctx.enter_context(tc.tile_pool(name="psAB", bufs=2, space="PSUM"))
            pall_p = ctx.enter_context(tc.tile_pool(name="pall", bufs=2, space="PSUM"))
            psM_p = ctx.enter_context(tc.tile_pool(name="psM", bufs=3, space="PSUM"))

            # ---- constants in SBUF ----
            FhT_t = const_p.tile([128, 128], F32R)   # [128, (hc2 x 64)]: cols hc*64
            FwR_t = const_p.tile([128, 64], F32R)
            FwI_t = const_p.tile([128, 64], F32R)
            for hc in range(2):
                nc.sync.dma_start(FhT_t[:, hc * 64:(hc + 1) * 64],
                                  d_FhT[hc * 128:(hc + 1) * 128, :])
                nc.sync.dma_start(FwR_t[:, hc * 32:(hc + 1) * 32],
                                  d_FwR[hc * 128:(hc + 1) * 128, :])
                nc.sync.dma_start(FwI_t[:, hc * 32:(hc + 1) * 32],
                                  d_FwI[hc * 128:(hc + 1) * 128, :])
            GhD_t = const_p.tile([64, H], F32R)
            nc.sync.dma_start(GhD_t[:], d_GhD[:])
            GwE_t = const_p.tile([32, W], F32R)
            nc.sync.dma_start(GwE_t[:], d_GwE[:])
            skipT_t = const_p.tile([64, L * C], F32R)   # [c, l*64+o]
            biasP_t = const_p.tile([64, L], F32)
            for li in range(L):
                nc.sync.dma_start(skipT_t[:, li * C:(li + 1) * C], d_skipT[li])
                nc.sync.dma_start(biasP_t[:, li:li + 1], d_biasP[li].unsqueeze(1))
            liftT_t = const_p.tile([CIN, C], F32R)
            nc.sync.dma_start(liftT_t[:], d_liftT[:])
            liftb_t = const_p.tile([64, 1], F32)
            nc.sync.dma_start(liftb_t[:], d_liftb[:].unsqueeze(1))
            projT_t = const_p.tile([C, C], F32R)
            nc.sync.dma_start(projT_t[:], d_projT[:])
            projb_t = const_p.tile([64, 1], F32)
            nc.sync.dma_start(projb_t[:], d_projb[:].unsqueeze(1))
            ident_t = const_p.tile([64, 64], F32)
            make_identity(nc, ident_t[:])

            # persistent working tensors
            XhT = [big_p.tile([128, 64 * C], F32R, tag=f"XhT{wc}", name=f"XhT{wc}") for wc in range(2)]
            RB = big_p.tile([128, NMODE * 4], F32)
            X3E = [big_p.tile([64, KX * 128], F32R, tag=f"X3E{s}", name=f"X3E{s}") for s in range(BS)]
            T_all = big_p.tile([32, C * H], F32R)
            pacc = big_p.tile([64, 256], F32)

            def xdram(l):
                """(src, dst) activation dram tensors for layer l (ping-pong)."""
                return (d_xA, d_xB) if l % 2 == 0 else (d_xB, d_xA)

            # ================= lift =================
            for s in range(BS):
                for i in range(HW // NPX):
                    gt = xt_p.tile([CIN, NPX], F32R, tag="gtile")
                    nc.sync.dma_start(gt[:], d_grid[s, :, i * NPX:(i + 1) * NPX])
                    ps = psM_p.tile([64, NPX], F32, tag="misc", name="psL")
                    nc.tensor.matmul(ps[:], liftT_t[:], gt[:], start=True, stop=True)
                    ot = ep_p.tile([64, NPX], F32R, tag="lift_o")
                    nc.scalar.activation(ot[:], ps[:], GELU, bias=liftb_t[:])
                    nc.sync.dma_start(d_xA[s, :, i * NPX:(i + 1) * NPX], ot[:])
            if debug_taps:
                for s in range(BS):
                    nc.sync.dma_start(taps['x0'][s], d_xA[s])

            # ================= layers =================
            for l in range(L):
                d_xin, d_xout = xdram(l)
                # ---------- stages A+B per sample ----------
                for s in range(BS):
                    xin_s = d_xin[s].rearrange("c (hc h w) -> c hc h w", hc=2, h=128)
                    for c in range(C):
                        xt = [xt_p.tile([128, W], F32R, tag="xA_t", name="xA_t") for _ in range(2)]
                        for hc in range(2):
                            nc.sync.dma_start(xt[hc][:], xin_s[c, hc])
                        for wc in range(2):
                            psA = psAB_p.tile([128, 64], F32, tag="ab", name="psA")
                            for hc in range(2):
                                nc.tensor.matmul(
                                    psA[:], xt[hc][:, wc * 128:(wc + 1) * 128],
                                    FhT_t[:, hc * 64:(hc + 1) * 64],
                                    start=(hc == 0), stop=(hc == 1))
                            # psA [128w, 64 kyr] -> XhT[wc][:, kyr*64 + c]
                            nc.vector.tensor_copy(
                                XhT[wc][:].rearrange("p (k c) -> p k c", k=64)[:, :, c],
                                psA[:])
                    # ---------- stage B for this sample ----------
                    for ky in range(KY):
                        psB = psAB_p.tile([64, 32], F32, tag="ab", name="psB")
                        step = 0
                        for rin in range(2):
                            rhs_t = FwR_t if rin == 0 else FwI_t
                            for wc in range(2):
                                kyr = rin * 32 + ky
                                nc.tensor.matmul(
                                    psB[:],
                                    XhT[wc][:, kyr * 64:(kyr + 1) * 64],
                                    rhs_t[:, wc * 32:(wc + 1) * 32],
                                    start=(step == 0), stop=(step == 3))
                                step += 1
                        # scatter into RB: col = mode*4 + var*2 + s ; mode = ky*16+kx
                        rbv = RB[:].rearrange("p (ky kx v) -> p ky kx v", ky=KY, kx=KX)
                        nc.vector.tensor_copy(rbv[0:64, ky, :, 0 * 2 + s], psB[:, 0:16])
                        nc.vector.tensor_scalar_mul(rbv[64:128, ky, :, 0 * 2 + s],
                                                    psB[:, 16:32], -1.0)
                        nc.vector.tensor_copy(rbv[0:64, ky, :, 1 * 2 + s], psB[:, 16:32])
                        nc.vector.tensor_copy(rbv[64:128, ky, :, 1 * 2 + s], psB[:, 0:16])

                if debug_taps and l == 0:
                    nc.sync.dma_start(taps['RB'][:], RB[:])

                # ---------- stage C (kx-major) + C->D transposes ----------
                # mode' = kxg*128 + ky*4 + kxl ; original mode = ky*16 + kxg*4 + kxl
                for kxg in range(4):
                    pall = pall_p.tile([64, 512], F32, tag="pall", name="pall")
                    for chl in range(8):           # 8 chunks of 16 modes per kxg
                        ch = kxg * 8 + chl
                        wsb = wsb_p.tile([128, WCHUNK * C], F32, tag="wsb")
                        nc.sync.dma_start(wsb[:], d_W2[l, ch])
                        for mi in range(WCHUNK):
                            mp = chl * WCHUNK + mi           # mode-in-group: ky*4+kxl
                            ky, kxl = mp // 4, mp % 4
                            mode = ky * KX + kxg * 4 + kxl   # original mode for RB col
                            out_ap = pall[:].rearrange(
                                "p (kxl s v q) -> p kxl v s q", kxl=4, s=2, v=2)[:, kxl, :, :, ky]
                            # cols: kxl*128 + s*64 + v*32 + ky ; n-order (v, s)
                            nc.tensor.matmul(out_ap,
                                             wsb[:, mi * C:(mi + 1) * C],
                                             RB[:, mode * 4:(mode + 1) * 4],
                                             start=True, stop=True)
                    for kxl in range(4):
                        kx = kxg * 4 + kxl
                        pcp = sb_p.tile([64, 128], F32, tag="pcp")
                        nc.vector.tensor_copy(pcp[:], pall[:, kxl * 128:(kxl + 1) * 128])
                        psT = psM_p.tile([128, 64], F32, tag="misc", name="psT")
                        nc.tensor.transpose(psT[:], pcp[:], ident_t[:])
                        # psT [128 = s*64+v*32+ky, 64 o]
                        for s in range(BS):
                            x3v = X3E[s][:].rearrange("p (v kx o) -> p v kx o", v=2, kx=KX)
                            nc.vector.tensor_copy(x3v[0:32, 0, kx, :], psT[s * 64:s * 64 + 32, :])
                            nc.vector.tensor_scalar_mul(x3v[32:64, 0, kx, :],
                                                        psT[s * 64 + 32:s * 64 + 64, :], -1.0)
                            nc.vector.tensor_copy(x3v[0:32, 1, kx, :], psT[s * 64 + 32:s * 64 + 64, :])
                            nc.vector.tensor_copy(x3v[32:64, 1, kx, :], psT[s * 64:s * 64 + 32, :])

                if debug_taps and l == 0:
                    for s in range(BS):
                        nc.sync.dma_start(taps['X3E'][s], X3E[s][:].bitcast(F32))

                # ---------- stages D+E per sample ----------
                for s in range(BS):
                    for o in range(C):
                        psD = psM_p.tile([32, H], F32, tag="misc", name="psD")
                        lhs = X3E[s][:].rearrange("p (m o) -> p m o", m=32)[:, :, o]
                        nc.tensor.matmul(psD[:], lhs, GhD_t[:], start=True, stop=True)
                        nc.vector.tensor_copy(
                            T_all[:, o * H:(o + 1) * H], psD[:])
                    if debug_taps and l == 0:
                        nc.sync.dma_start(taps['Tp'][s].rearrange("c x h -> x c h"),
                                          T_all[:].bitcast(F32).rearrange("p (c h) -> p c h", c=C))

                    xin_r = d_xin[s].rearrange("c (hp q) -> c hp q", q=2 * W)
                    xout_r = d_xout[s].rearrange("c (hp q) -> c hp q", q=2 * W)
                    tav = T_all[:].rearrange("p (o h) -> p h o", o=C)
                    for hp in range(H // 2):
                        xt = ep_p.tile([64, 2 * W], F32R, tag="xE_t")
                        nc.sync.dma_start(xt[:], xin_r[:, hp])
                        psY = psM_p.tile([64, 2 * W], F32, tag="misc", name="psY")
                        nc.tensor.matmul(psY[:], skipT_t[:, l * C:(l + 1) * C], xt[:],
                                         start=True, stop=False)
                        for j in range(2):
                            nc.tensor.matmul(psY[:, j * W:(j + 1) * W],
                                             tav[:, hp * 2 + j, :], GwE_t[:],
                                             start=False, stop=True)
                        gt = ep_p.tile([64, 2 * W], F32R, tag="gE_t")
                        nc.scalar.activation(gt[:], psY[:], GELU, bias=biasP_t[:, l:l + 1])
                        nc.vector.tensor_add(gt[:], gt[:], xt[:])
                        nc.sync.dma_start(xout_r[:, hp], gt[:])

                if debug_taps and l == 0:
                    for s in range(BS):
                        nc.sync.dma_start(taps['x1'][s], d_xout[s])

            # ================= proj + pool =================
            d_xfin = xdram(L)[0]
            for s in range(BS):
                for i in range(HW // NPX):
                    xt = xt_p.tile([C, NPX], F32R, tag="xP_t")
                    nc.sync.dma_start(xt[:], d_xfin[s, :, i * NPX:(i + 1) * NPX])
                    ps = psM_p.tile([64, NPX], F32, tag="misc", name="psP")
                    nc.tensor.matmul(ps[:], projT_t[:], xt[:], start=True, stop=True)
                    ot = ep_p.tile([64, NPX], F32, tag="proj_o")
                    nc.scalar.activation(ot[:], ps[:], GELU, bias=projb_t[:],
                                         accum_out=pacc[:, s * 128 + i:s * 128 + i + 1])

            # ================= heads =================
            pooled = sb_p.tile([64, BS], F32, tag="pooled")
            for s in range(BS):
                nc.vector.tensor_reduce(
                    pooled[:, s:s + 1], pacc[:, s * 128:(s + 1) * 128],
                    axis=mybir.AxisListType.X, op=mybir.AluOpType.add)
            xh = sb_p.tile([HEAD_IN, BS], F32, tag="xh")
            nc.vector.tensor_copy(xh[0:C, :], pooled[:])
            nc.sync.dma_start(xh[C:C + ENV, :], d_env[:].rearrange("s p -> p s"))
            nc.sync.dma_start(xh[C + ENV:HEAD_IN, :], d_d1d[:].rearrange("s p -> p s"))

            def head(w1d, w2d, w3d, bds, d_out, width3):
                w1t = sb_p.tile([HEAD_IN, 128], F32, tag="hw1")
                nc.sync.dma_start(w1t[:], w1d[:])
                w2t = sb_p.tile([128, 64], F32, tag="hw2")
                nc.sync.dma_start(w2t[:], w2d[:])
                w3t = sb_p.tile([64, width3], F32, tag="hw3")
                nc.sync.dma_start(w3t[:], w3d[:])
                b1t = sb_p.tile([128, 1], F32, tag="hb1")
                nc.sync.dma_start(b1t[:], bds[0][:].unsqueeze(1))
                b2t = sb_p.tile([64, 1], F32, tag="hb2")
                nc.sync.dma_start(b2t[:], bds[1][:].unsqueeze(1))
                b3t = sb_p.tile([width3, 1], F32, tag="hb3")
                nc.sync.dma_start(b3t[:], bds[2][:].unsqueeze(1))
                p1 = psM_p.tile([128, BS], F32, tag="misc", name="p1")
                nc.tensor.matmul(p1[:], w1t[:], xh[:], start=True, stop=True)
                h1 = sb_p.tile([128, BS], F32, tag="hh1")
                nc.scalar.activation(h1[:], p1[:], GELU, bias=b1t[:])
                p2 = psM_p.tile([64, BS], F32, tag="misc", name="p2")
                nc.tensor.matmul(p2[:], w2t[:], h1[:], start=True, stop=True)
                h2 = sb_p.tile([64, BS], F32, tag="hh2")
                nc.scalar.activation(h2[:], p2[:], GELU, bias=b2t[:])
                p3 = psM_p.tile([width3, BS], F32, tag="misc", name="p3")
                nc.tensor.matmul(p3[:], w3t[:], h2[:], start=True, stop=True)
                h3 = sb_p.tile([width3, BS], F32, tag="hh3")
                nc.scalar.activation(h3[:], p3[:], IDENT, bias=b3t[:])
                nc.sync.dma_start(d_out[:].rearrange("s o -> o s"), h3[:])

            head(d_dw1T, d_dw2T, d_dw3T, d_db, d_odir, 8)
            head(d_iw1T, d_iw2T, d_iw3T, d_ib, d_oint, 4)

    nc.compile()
    return nc, taps


def _get_compiled(debug_taps=False):
    key = ('prog', debug_taps)
    if key not in _CACHE:
        _CACHE[key] = build_program(debug_taps)
    return _CACHE[key]


def make_in_maps(inputs, ct):
    grid = np.asarray(inputs['grid'], np.float32).reshape(B, CIN, HW)
    env = np.asarray(inputs['env'], np.float32)
    d1d = np.asarray(inputs['d1d'], np.float32)
    maps = []
    for core in range(NCORES):
        sl = slice(core * BS, (core + 1) * BS)
        m = {
            'grid': np.ascontiguousarray(grid[sl]),
            'env': np.ascontiguousarray(env[sl]),
            'd1d': np.ascontiguousarray(d1d[sl]),
            'FhT': ct['FhT'], 'FwR': ct['FwR'], 'FwI': ct['FwI'],
            'W2': ct['W2'].reshape(L, NMODE // WCHUNK, 128, WCHUNK * C),
            'GhD': ct['GhD'], 'GwE': ct['GwE'],
            'skip_wT': ct['skip_wT'], 'biasP': ct['biasP'],
            'lift_wT': ct['lift_wT'], 'lift_b': ct['lift_b'],
            'proj_wT': ct['proj_wT'], 'proj_b': ct['proj_b'],
            'dw1T': ct['dw1T'], 'dw2T': ct['dw2T'], 'dw3T': ct['dw3T'],
            'iw1T': ct['iw1T'], 'iw2T': ct['iw2T'], 'iw3T': ct['iw3T'],
            'db1': ct['db1'], 'db2': ct['db2'], 'db3': ct['db3'],
            'ib1': ct['ib1'], 'ib2': ct['ib2'], 'ib3': ct['ib3'],
        }
        maps.append(m)
    return maps


def run_on_device(inputs, debug_taps=False, trace=False):
    from concourse.bass_utils import run_bass_kernel_spmd
    nc, taps = _get_compiled(debug_taps)
    ct = precompute_consts({k: np.asarray(v) for k, v in inputs.items()})
    in_maps = make_in_maps(inputs, ct)
    res = run_bass_kernel_spmd(nc, in_maps, core_ids=list(range(NCORES)), trace=trace)
    out_dir = np.concatenate([res.results[i]['out_dir'] for i in range(NCORES)], axis=0)
    out_int = np.concatenate([res.results[i]['out_int'] for i in range(NCORES)], axis=0)
    return (out_dir, out_int), res


def kernel(**inputs):
    (out_dir, out_int), _ = run_on_device(inputs)
    return out_dir.astype(np.float32), out_int.astype(np.float32)
